# revision 15
# baseline (speedup 1.0000x reference)
"""GAT (2-layer, 8-head) Bass kernel for 8 Trainium2 NeuronCores — v3.

vs v2 (365.8us -> 344.1us): layer-2 attention packs [X2p-aug | pad |
X2m-aug | pad] into one M=128 stationary so a single mpt-moving chain
yields Pp2 and Qm2 together, plus one Sm2 chain against the raw mask
(96 -> 64 PE passes); positive r1o broadcast with an Sm2-Qm2 combine
(sm2 staged through SBUF since ops may read only one PSUM operand);
mask DMA split into 4 chunks on the SP queue.

F layout: host orders wa columns [f1 x8 | f2 x8] so f2 slices are
contiguous.
"""
import sys

sys.path.insert(0, "/opt/trn_rl_repo")

import numpy as np
import ml_dtypes

import concourse.bass as bass
import concourse.bacc as bacc
import concourse.tile as tile
import concourse.mybir as mybir
from concourse.bass_utils import run_bass_kernel_spmd

F32 = mybir.dt.float32
BF16 = mybir.dt.bfloat16
AF = mybir.ActivationFunctionType
ALU = mybir.AluOpType
AX = mybir.AxisListType

NCORES = 8
N = 4096
FIN = 256
HID = 64
H = 8
NC = 41
ROWS = N // NCORES   # 512
JT = N // 128        # 32 j tiles
IT = ROWS // 128     # 4
AUG = HID + 1        # 65
AUG2 = NC + 1        # 42
PAY = 1 + NC         # gather payload: f2 | 41 classes; ones col appended after
HJT = JT // 2        # half-head X-prep granularity
ALPHA = 0.2

A_SET = (2, 5)       # ACT-engine (exp) heads

_CACHED_NC = None


def _build(ablate=()):
    nc = bacc.Bacc("TRN2", target_bir_lowering=False, debug=False,
                   num_devices=NCORES)

    def dram_in(name, shape, dt=BF16):
        return nc.dram_tensor(name, list(shape), dt, kind="ExternalInput").ap()

    xT = dram_in("xT", [128, 2, N])
    xrT = dram_in("xrT", [128, 2, ROWS])
    wcat = dram_in("wcat", [128, 2, H * HID])
    wa = dram_in("wa", [128, 2, 2 * H])
    wout = dram_in("wout", [128, 4, NC])
    woa1 = dram_in("woa1", [128, 4])
    a2b = dram_in("a2b", [128, NC])
    csum = dram_in("csum", [128, NC], F32)
    ident = dram_in("ident", [128, NC], F32)
    consts = dram_in("consts", [128, 8], F32)
    maskT = dram_in("maskT", [128, JT, ROWS])
    rsel16_d = dram_in("rsel16", [2 * H, H, 128])
    rsel8_d = dram_in("rsel8", [H, H, 128])
    half2_d = dram_in("half2", [1, 2, 128], F32)
    out = nc.dram_tensor("out", [NC, ROWS], F32, kind="ExternalOutput").ap()

    with tile.TileContext(nc) as tc:
        with (
            tc.tile_pool(name="dram", bufs=1, space="DRAM") as dpool,
            tc.tile_pool(name="const", bufs=1) as cp,
            tc.tile_pool(name="big", bufs=1) as bigp,
            tc.tile_pool(name="xt", bufs=3) as xtp,
            tc.tile_pool(name="xp", bufs=2) as xpp,
            tc.tile_pool(name="mp", bufs=2) as mpp,
            tc.tile_pool(name="mpa", bufs=4) as mpa,
            tc.tile_pool(name="p4", bufs=1) as p4p,
            tc.tile_pool(name="work", bufs=1) as wp,
            tc.tile_pool(name="tl", bufs=1) as tlp,
            tc.tile_pool(name="psS", bufs=1, space="PSUM") as psS,
            tc.tile_pool(name="psB", bufs=2, space="PSUM") as psB,
            tc.tile_pool(name="psW", bufs=2, space="PSUM") as psW,
            tc.tile_pool(name="psX", bufs=1, space="PSUM") as psX,
        ):
            # ---------------- stage 0: loads + consts ----------------
            xrT_sb = cp.tile([128, 2, ROWS], BF16)
            nc.sync.dma_start(out=xrT_sb[:], in_=xrT[:])
            wcat_sb = cp.tile([128, 2, H * HID], BF16)
            nc.sync.dma_start(out=wcat_sb[:], in_=wcat[:])
            wa_sb = cp.tile([128, 2, 2 * H], BF16)
            nc.sync.dma_start(out=wa_sb[:], in_=wa[:])
            wout_sb = cp.tile([128, 4, NC], BF16)
            nc.sync.dma_start(out=wout_sb[:], in_=wout[:])
            woa1_sb = cp.tile([128, 4], BF16)
            nc.sync.dma_start(out=woa1_sb[:], in_=woa1[:])
            a2b_sb = cp.tile([128, NC], BF16)
            nc.sync.dma_start(out=a2b_sb[:], in_=a2b[:])
            csum_sb = cp.tile([128, NC], F32)
            nc.sync.dma_start(out=csum_sb[:], in_=csum[:])
            consts_sb = cp.tile([128, 8], F32)
            nc.sync.dma_start(out=consts_sb[:], in_=consts[:])
            mask_sb = bigp.tile([128, JT, ROWS], BF16)
            for mq in range(4):
                s = slice(mq * (JT // 4), (mq + 1) * (JT // 4))
                nc.sync.dma_start(out=mask_sb[:, s, :], in_=maskT[:, s, :])

            ones_sb = cp.tile([1, 128], BF16)
            nc.gpsimd.memset(ones_sb[:], 1.0)
            # row-broadcast selector stationaries (host consts)
            rsel16 = cp.tile([2 * H, H, 128], BF16)
            nc.sync.dma_start(out=rsel16[:], in_=rsel16_d[:])
            rsel8 = cp.tile([H, H, 128], BF16)
            nc.sync.dma_start(out=rsel8[:], in_=rsel8_d[:])
            half2 = cp.tile([1, 2, 128], F32)
            nc.sync.dma_start(out=half2[:], in_=half2_d[:])

            whaug = bigp.tile([128, JT, H, AUG], BF16)
            nc.gpsimd.memset(whaug[:, :, :, HID:AUG], 1.0)


            # ---------------- stage A: f / F prep ----------------
            pfmy = psS.tile([2 * H, ROWS], F32, tag="s")
            for kt in range(2):
                nc.tensor.matmul(pfmy[:], wa_sb[:, kt, :], xrT_sb[:, kt, :],
                                 start=(kt == 0), stop=(kt == 1))
            fmy_bf = cp.tile([2 * H, ROWS], BF16)
            nc.vector.tensor_copy(fmy_bf[:], pfmy[:])
            rmy = wp.tile([H, ROWS], BF16, tag="dt0")
            nc.scalar.activation(rmy[:], pfmy[0:H, :], AF.Exp, scale=-(1.0 - ALPHA))
            rmy_nb = cp.tile([H, ROWS], BF16)
            nc.vector.tensor_scalar(rmy_nb[:], rmy[:], -1.0, None, op0=ALU.mult)

            # +f1 broadcasts for all heads
            f1b_all = cp.tile([128, H, ROWS], BF16)
            for h in range(H):
                pb = psX.tile([128, ROWS], F32, tag="bc")
                nc.tensor.matmul(pb[:], rsel16[:, h, :], fmy_bf[:],
                                 start=True, stop=True)
                nc.scalar.activation(f1b_all[:, h, :], pb[:], AF.Copy)

            # fused F + Wh loop: one xt load per tile
            F_sb = cp.tile([128, JT, 2 * H], F32)
            EJ = cp.tile([128, JT, H], BF16)
            GJ = cp.tile([128, JT, H], BF16)
            for it in range(JT):
                xt_t = xtp.tile([128, 2, 128], BF16, tag="xt")
                nc.sync.dma_start(out=xt_t[:], in_=xT[:, :, it * 128:(it + 1) * 128])
                pf = psS.tile([128, 2 * H], F32, tag="s")
                for kt in range(2):
                    nc.tensor.matmul(pf[:], xt_t[:, kt, :],
                                     wa_sb[:, kt, :], start=(kt == 0), stop=(kt == 1))
                pwh = psW.tile([128, H * HID], F32, tag="wh")
                for kt in range(2):
                    nc.tensor.matmul(pwh[:], xt_t[:, kt, :],
                                     wcat_sb[:, kt, :], start=(kt == 0), stop=(kt == 1))
                nc.vector.tensor_copy(F_sb[:, it, :], pf[:])
                nc.scalar.copy(whaug[:, it, :, 0:HID],
                               pwh.rearrange("p (h d) -> p h d", h=H))
                if it == HJT - 1 or it == JT - 1:
                    s = slice(0, HJT) if it == HJT - 1 else slice(HJT, JT)
                    nc.scalar.activation(EJ[:, s, :], F_sb[:, s, H:2 * H], AF.Exp)
                    nc.scalar.activation(GJ[:, s, :], F_sb[:, s, H:2 * H], AF.Exp,
                                         scale=ALPHA)

            negF2 = cp.tile([128, JT, H], F32)
            nc.vector.tensor_scalar(negF2[:], F_sb[:, :, H:2 * H], -1.0, None,
                                    op0=ALU.mult)

            # r1b for B-heads (needed from first combine on)
            r1b_all = cp.tile([128, H, ROWS], BF16)
            for h in range(H):
                if h in A_SET:
                    continue
                pb = psX.tile([128, ROWS], F32, tag="bc")
                nc.tensor.matmul(pb[:], rsel8[:, h, :], rmy_nb[:],
                                 start=True, stop=True)
                nc.scalar.activation(r1b_all[:, h, :], pb[:], AF.Copy)

            # ---------------- A-head exp streams (ACT + GP) ----------------
            pmtA = {h: [] for h in A_SET}
            for h in A_SET:
                for g in range(JT // 4):
                    j0 = g * 4
                    pt4 = p4p.tile([128, 4, ROWS], BF16, tag="pt4")
                    for q in range(4):
                        jt = j0 + q
                        et = wp.tile([128, ROWS], F32, tag="et")
                        nc.scalar.activation(et[:], f1b_all[:, h, :], AF.Prelu,
                                             bias=F_sb[:, jt, H + h:H + h + 1],
                                             alpha=ALPHA)
                        nc.scalar.activation(pt4[:, q, :], et[:], AF.Exp)
                    pmt_g = mpa.tile([128, 4, ROWS], BF16, tag="pmA")
                    nc.vector.tensor_tensor(pmt_g[:], pt4[:],
                                            mask_sb[:, j0:j0 + 4, :], op=ALU.mult)
                    pmtA[h].append(pmt_g)

            # ---------------- stage B: heads (pipelined) ----------------
            xcraw = cp.tile([128, IT, ROWS], BF16)
            xcT = cp.tile([128, IT, ROWS], BF16)
            rcp1 = {}
            pending = []
            done_heads = set()

            def emit_tail(q):
                prb = psX.tile([128, ROWS], F32, tag="bc")
                nc.tensor.matmul(prb[:], half2[0:1, 0, :], rcp1[2 * q][:],
                                 start=True, stop=False)
                nc.tensor.matmul(prb[:], half2[0:1, 1, :], rcp1[2 * q + 1][:],
                                 start=False, stop=True)
                hn = tlp.tile([128, ROWS], BF16, tag="hn")
                nc.vector.tensor_tensor(hn[:], xcraw[:, q, :], prb[:], op=ALU.mult)
                tm = tlp.tile([128, ROWS], BF16, tag="tm")
                nc.vector.tensor_scalar(tm[:], hn[:], 0.0, None, op0=ALU.min)
                te = wp.tile([128, ROWS], BF16, tag="nlse")
                nc.scalar.activation(te[:], tm[:], AF.Exp)
                nc.vector.scalar_tensor_tensor(xcT[:, q, :], hn[:], 0.0, te[:],
                                               op0=ALU.max, op1=ALU.add)

            def finish_head(h):
                done_heads.add(h)

            def emit_combine(h, pp, pm):
                if pm is None:
                    nc.vector.tensor_copy(
                        xcraw[(h % 2) * HID:(h % 2) * HID + HID, h // 2, :],
                        pp[0:HID, :])
                    dtmp = wp.tile([1, ROWS], F32, tag=f"dt{h % 2}")
                    nc.vector.tensor_copy(dtmp[:], pp[HID:AUG, :])
                else:
                    t2 = wp.tile([AUG, ROWS], F32, tag="t2")
                    nc.vector.tensor_tensor(t2[:], pm[:], r1b_all[0:AUG, h, :],
                                            op=ALU.mult)
                    nc.vector.tensor_tensor(
                        xcraw[(h % 2) * HID:(h % 2) * HID + HID, h // 2, :],
                        pp[0:HID, :], t2[0:HID, :], op=ALU.add)
                    dtmp = wp.tile([1, ROWS], F32, tag=f"dt{h % 2}")
                    nc.vector.tensor_tensor(dtmp[:], pp[HID:AUG, :],
                                            t2[HID:AUG, :], op=ALU.add)
                r = wp.tile([1, ROWS], F32, tag=f"rc{h % 2}")
                nc.vector.reciprocal_approx_fast(r[:], dtmp[:])
                rcp1[h] = r
                finish_head(h)
            for h in range(H):
                if h in A_SET:
                    pa = psB.tile([AUG, ROWS], F32, tag="pp")
                    for g in range(JT // 4):
                        for q in range(4):
                            jt = g * 4 + q
                            nc.tensor.matmul(pa[:], whaug[:, jt, h, :],
                                             pmtA[h][g][:, q, :],
                                             start=(jt == 0), stop=(jt == JT - 1))
                    while pending:
                        emit_combine(*pending.pop(0))
                    pending.append((h, pa, None))
                    continue
                Xh = {}

                def xprep(half):
                    s = slice(half * HJT, (half + 1) * HJT)
                    Xp = xpp.tile([128, HJT, AUG], BF16, tag=f"Xp{half}")
                    Xm = xpp.tile([128, HJT, AUG], BF16, tag=f"Xm{half}")
                    Xmn = xpp.tile([128, HJT, AUG], BF16, tag=f"Xmn{half}")
                    eb = EJ[:, s, h:h + 1].broadcast_to([128, HJT, AUG])
                    gb = GJ[:, s, h:h + 1].broadcast_to([128, HJT, AUG])
                    nc.vector.tensor_tensor(Xp[:], whaug[:, s, h, :], eb, op=ALU.mult)
                    nc.vector.tensor_tensor(Xm[:], whaug[:, s, h, :], gb, op=ALU.mult)
                    nc.vector.tensor_scalar(Xmn[:], Xm[:], -1.0, None, op0=ALU.mult)
                    Xh[half] = (Xp, Xm, Xmn)

                xprep(0)
                pp = psB.tile([AUG, ROWS], F32, tag="pp")
                pm = psB.tile([AUG, ROWS], F32, tag="pm")
                for g in range(JT // 4):
                    j0 = g * 4
                    if g == 2:
                        xprep(1)
                    Xp, Xm, Xmn = Xh[g // 4]
                    prt = mpp.tile([128, 4, ROWS], BF16, tag="prt")
                    for q in range(4):
                        jt = j0 + q
                        nc.vector.tensor_scalar(prt[:, q, :], f1b_all[:, h, :],
                                                negF2[:, jt, h:h + 1],
                                                None, op0=ALU.is_ge)
                    mpt = mpp.tile([128, 4, ROWS], BF16, tag="mpt")
                    nc.vector.tensor_tensor(mpt[:], prt[:],
                                            mask_sb[:, j0:j0 + 4, :], op=ALU.mult)
                    for q in range(4):
                        jt = j0 + q
                        jl = jt % HJT
                        nc.tensor.matmul(pp[:], Xp[:, jl, :], mpt[:, q, :],
                                         start=(jt == 0), stop=(jt == JT - 1))
                        nc.tensor.matmul(pm[:], Xmn[:, jl, :], mask_sb[:, jt, :],
                                         start=(jt == 0), stop=False)
                        nc.tensor.matmul(pm[:], Xm[:, jl, :], mpt[:, q, :],
                                         start=False, stop=(jt == JT - 1))
                while pending:
                    emit_combine(*pending.pop(0))
                pending.append((h, pp, pm))
            while pending:
                emit_combine(*pending.pop(0))
            for q4 in range(IT):
                emit_tail(q4)

            # ---------------- stage C: Wh2 + f1o/f2 + gather ----------------
            pay = cp.tile([128, IT, PAY], BF16)
            for it in range(IT):
                pw2 = psW.tile([128, NC], F32, tag="wh")
                for kt in range(IT):
                    nc.tensor.matmul(pw2[:], xcT[:, kt, it * 128:(it + 1) * 128],
                                     wout_sb[:, kt, :], start=(kt == 0), stop=(kt == 3))
                nc.vector.scalar_tensor_tensor(pay[:, it, 1:1 + NC], pw2[:], 0.0,
                                               csum_sb[:], op0=ALU.add, op1=ALU.subtract)
            pf1o = psS.tile([1, ROWS], F32, tag="s")
            for kt in range(IT):
                nc.tensor.matmul(pf1o[:], woa1_sb[:, kt:kt + 1], xcT[:, kt, :],
                                 start=(kt == 0), stop=(kt == IT - 1))
            R1o_pb = cp.tile([1, ROWS], BF16)
            nc.scalar.activation(R1o_pb[:], pf1o[:], AF.Exp, scale=-(1.0 - ALPHA),
                                 bias=consts_sb[0:1, 2:3])
            f1o_bf = cp.tile([1, ROWS], BF16)
            nc.scalar.activation(f1o_bf[:], pf1o[:], AF.Identity,
                                 bias=consts_sb[0:1, 0:1])

            t41 = wp.tile([128, IT, NC], F32, tag="t41")
            nc.vector.tensor_tensor(
                t41[:], pay[:, :, 1:1 + NC],
                a2b_sb[:].rearrange("p (o c) -> p o c", o=1).broadcast_to([128, IT, NC]),
                op=ALU.mult)
            with nc.allow_low_precision(reason="f2 payload rounds to bf16 anyway"):
                nc.vector.reduce_sum(pay[:, :, 0:1], t41[:], axis=AX.X)

            ag_in = dpool.tile([128, IT, PAY], BF16)
            nc.gpsimd.dma_start(ag_in[:], pay[:])
            ag_out = dpool.tile([NCORES, 128, IT, PAY], BF16)
            nc.gpsimd.collective_compute(
                "AllGather", ALU.bypass,
                replica_groups=[list(range(NCORES))],
                ins=[ag_in.opt()], outs=[ag_out.opt()],
            )
            wh2f = cp.tile([128, JT, PAY + 1], BF16)
            nc.gpsimd.memset(wh2f[:, :, PAY:PAY + 1], 1.0)
            for r in range(NCORES):
                nc.sync.dma_start(out=wh2f[:, r * IT:(r + 1) * IT, 0:PAY],
                                  in_=ag_out[r])

            # ---------------- stage D: layer-2 attention ----------------
            f1b2 = cp.tile([128, ROWS], BF16)
            pb2 = psX.tile([128, ROWS], F32, tag="bc")
            nc.tensor.matmul(pb2[:], ones_sb[:], f1o_bf[:], start=True, stop=True)
            nc.scalar.activation(f1b2[:], pb2[:], AF.Copy)

            nf2t = cp.tile([128, JT, 1], F32)
            nc.vector.tensor_scalar(nf2t[:], wh2f[:, :, 0:1], -1.0, None, op0=ALU.mult)
            E2o = cp.tile([128, JT, 1], BF16)
            nc.scalar.activation(E2o[:], wh2f[:, :, 0:1], AF.Exp)
            G2o = cp.tile([128, JT, 1], BF16)
            nc.scalar.activation(G2o[:], wh2f[:, :, 0:1], AF.Exp, scale=ALPHA)

            # packed stationary: [X2p-aug | pad | X2m-aug | pad] so one
            # mpt-moving pass yields Pp2 (partitions 0:42) and Qm2 (64:106)
            X2cat = cp.tile([128, JT, 2, 64], BF16)
            nc.gpsimd.memset(X2cat[:, :, :, AUG2:64], 0.0)
            w2s = wh2f[:, :, 1:1 + AUG2]
            nc.vector.tensor_tensor(X2cat[:, :, 0, 0:AUG2], w2s,
                                    E2o.broadcast_to([128, JT, AUG2]), op=ALU.mult)
            nc.vector.tensor_tensor(X2cat[:, :, 1, 0:AUG2], w2s,
                                    G2o.broadcast_to([128, JT, AUG2]), op=ALU.mult)

            pq2 = psB.tile([128, ROWS], F32, tag="pp")
            sm2 = psB.tile([AUG2, ROWS], F32, tag="pm")
            for g in range(JT // 4):
                j0 = g * 4
                prt = mpp.tile([128, 4, ROWS], BF16, tag="prt")
                for q in range(4):
                    jt = j0 + q
                    nc.vector.tensor_scalar(prt[:, q, :], f1b2[:],
                                            nf2t[:, jt, :], None, op0=ALU.is_ge)
                mpt = mpp.tile([128, 4, ROWS], BF16, tag="mpt")
                nc.vector.tensor_tensor(mpt[:], prt[:],
                                        mask_sb[:, j0:j0 + 4, :], op=ALU.mult)
                for q in range(4):
                    jt = j0 + q
                    nc.tensor.matmul(pq2[:], X2cat[:, jt, :, :], mpt[:, q, :],
                                     start=(jt == 0), stop=(jt == JT - 1))
                    nc.tensor.matmul(sm2[:], X2cat[:, jt, 1, 0:AUG2],
                                     mask_sb[:, jt, :],
                                     start=(jt == 0), stop=(jt == JT - 1))
            pbr = psX.tile([128, ROWS], F32, tag="bc")
            nc.tensor.matmul(pbr[:], ones_sb[:], R1o_pb[:], start=True, stop=True)
            r1b2 = cp.tile([128, ROWS], BF16)
            nc.scalar.activation(r1b2[:], pbr[:], AF.Copy)
            s2sb = wp.tile([AUG2, ROWS], F32, tag="te2")
            nc.scalar.copy(s2sb[:], sm2[:])
            u2 = wp.tile([AUG2, ROWS], F32, tag="et")
            nc.vector.tensor_tensor(u2[:], s2sb[:], pq2[64:64 + AUG2, :],
                                    op=ALU.subtract)
            t22 = wp.tile([AUG2, ROWS], F32, tag="t2")
            nc.vector.tensor_tensor(t22[:], u2[:], r1b2[0:AUG2, :], op=ALU.mult)
            hs2 = wp.tile([AUG2, ROWS], F32, tag="et")
            nc.vector.tensor_tensor(hs2[:], pq2[0:AUG2, :], t22[:], op=ALU.add)
            srow2 = wp.tile([1, ROWS], F32, tag="srow2")
            nc.sync.dma_start(out=srow2[:], in_=hs2[NC:AUG2, :])
            rr2 = wp.tile([1, ROWS], F32, tag="t2")
            nc.vector.reciprocal_approx_fast(rr2[:], srow2[:])
            rr2b = wp.tile([1, ROWS], BF16, tag="rr2b")
            nc.vector.tensor_copy(rr2b[:], rr2[:])
            prb2 = psX.tile([128, ROWS], F32, tag="bc")
            nc.tensor.matmul(prb2[:], ones_sb[:], rr2b[:], start=True, stop=True)
            zn = wp.tile([NC, ROWS], F32, tag="t41")
            tm2 = wp.tile([NC, ROWS], F32, tag="t2")
            te2 = wp.tile([NC, ROWS], F32, tag="te2")
            zel = wp.tile([NC, ROWS], F32, tag="zel")
            for half in range(2):
                hv = slice(half * (ROWS // 2), (half + 1) * (ROWS // 2))
                nc.vector.tensor_tensor(zn[:, hv], hs2[0:NC, hv], prb2[0:NC, hv],
                                        op=ALU.mult)
                nc.vector.tensor_scalar(tm2[:, hv], zn[:, hv], 0.0, None, op0=ALU.min)
                nc.scalar.activation(te2[:, hv], tm2[:, hv], AF.Exp)
                nc.vector.scalar_tensor_tensor(zel[:, hv], zn[:, hv], 0.0, te2[:, hv],
                                               op0=ALU.max, op1=ALU.add)

            # ---------------- stage E: log_softmax (matmul form) ----------------
            onescol = cp.tile([NC, 1], F32)
            nc.gpsimd.memset(onescol[:], 1.0)
            ez = wp.tile([NC, ROWS], F32, tag="t41")
            nc.scalar.activation(ez[:], zel[:], AF.Exp)
            psum1 = psS.tile([1, ROWS], F32, tag="s")
            nc.tensor.matmul(psum1[:], onescol[:], ez[:], start=True, stop=True)
            lse = wp.tile([1, ROWS], F32, tag="dt0")
            nc.scalar.activation(lse[:], psum1[:], AF.Ln)
            nlse = wp.tile([1, ROWS], BF16, tag="nlse")  # shares te tag
            nc.vector.tensor_scalar(nlse[:], lse[:], -1.0, None, op0=ALU.mult)
            plseb = psW.tile([NC, ROWS], F32, tag="wh")
            nc.tensor.matmul(plseb[:], ones_sb[0:1, 0:NC], nlse[:],
                             start=True, stop=True)
            zf = wp.tile([NC, ROWS], F32, tag="t2")
            nc.vector.tensor_tensor(zf[:], zel[:], plseb[:], op=ALU.add)
            nc.sync.dma_start(out=out[:], in_=zf[:])

    nc.compile()
    return nc


def _host_prep(x, adj, W, a, W_out, a_out):
    bf16 = ml_dtypes.bfloat16
    f32 = np.float32
    x = np.asarray(x, f32)
    W = np.asarray(W, f32)
    a = np.asarray(a, f32)
    W_out = np.asarray(W_out, f32)
    a_out = np.asarray(a_out, f32)

    def pk(arr, kt):  # [kt*128, M] -> [128, kt, M]
        return np.ascontiguousarray(
            arr.reshape(kt, 128, *arr.shape[1:]).transpose(1, 0, *range(2, arr.ndim + 1)))

    xT = pk(np.ascontiguousarray(x.T), 2).astype(bf16)
    wcat = pk(np.concatenate(list(W), axis=1), 2).astype(bf16)
    WA = np.zeros((FIN, 2 * H), f32)
    for h in range(H):
        WA[:, h] = W[h] @ a[h, :HID]          # f1 block
        WA[:, H + h] = W[h] @ a[h, HID:]      # f2 block
    wa = pk(WA, 2).astype(bf16)
    wout = pk(W_out, 4).astype(bf16)
    Woa1 = W_out @ a_out[:NC]
    woa1 = np.ascontiguousarray(Woa1.reshape(4, 128).T).astype(bf16)
    s = float(Woa1.sum())
    a2b = np.ascontiguousarray(np.broadcast_to(a_out[NC:], (128, NC))).astype(bf16)
    csum = np.ascontiguousarray(np.broadcast_to(W_out.sum(0), (128, NC))).astype(f32)
    ident = np.eye(128, NC, dtype=f32)
    consts = np.zeros((128, 8), f32)
    consts[:, 0] = -s
    consts[:, 2] = (1.0 - ALPHA) * s

    rsel16 = np.zeros((2 * H, H, 128), bf16)
    rsel8 = np.zeros((H, H, 128), bf16)
    for h in range(H):
        rsel16[h, h, :] = 1
        rsel8[h, h, :] = 1
    half2 = np.zeros((1, 2, 128), f32)
    half2[0, 0, 0:HID] = 1
    half2[0, 1, HID:128] = 1
    shared = dict(xT=xT, wcat=wcat, wa=wa, wout=wout, woa1=woa1, a2b=a2b,
                  csum=csum, ident=ident, consts=consts,
                  rsel16=rsel16, rsel8=rsel8, half2=half2)
    in_maps = []
    for c in range(NCORES):
        rows = slice(c * ROWS, (c + 1) * ROWS)
        mT = (np.asarray(adj[rows]).T > 0).astype(bf16)
        mT = np.ascontiguousarray(mT.reshape(JT, 128, ROWS).transpose(1, 0, 2))
        xr = pk(np.ascontiguousarray(x[rows].T), 2).astype(bf16)
        in_maps.append({**shared, "maskT": mT, "xrT": xr})
    return in_maps


def kernel(x, adj, W, a, W_out, a_out):
    global _CACHED_NC
    if _CACHED_NC is None:
        _CACHED_NC = _build()
    in_maps = _host_prep(x, adj, W, a, W_out, a_out)
    res = run_bass_kernel_spmd(_CACHED_NC, in_maps, list(range(NCORES)))
    out = np.concatenate([res.results[c]["out"].T for c in range(NCORES)], axis=0)
    return out.astype(np.float32)

